# revision 30
# baseline (speedup 1.0000x reference)
"""Causal multi-head attention (B=2, S=2048, D=1024, H=16) on 8 NeuronCores.

Sharding: head-parallel. Core c owns heads {2c, 2c+1} = a 128-wide slice of
the q/k/v projection output dims and of wo's input dim. Each core computes
attention for its 2 heads over both batch elements and a full-size partial
of the final projection; the host sums the 8 partials.

Design (v2):
- fp16 end-to-end (HBM tensors, SBUF operands; PSUM stays f32).
- scores computed transposed (scoresT[k, q]) so softmax probs come out
  k-partitioned and feed attn@v directly; a ones-column in the v tile emits
  the softmax denominators from the same matmul.
- causal mask applied post-exp on the (idle) Pool engine via affine_select.
- v tiles transposed via the DMA XBAR (dma_start_transpose), not PE.
- software-pipelined attention: av(kt-1) emitted after scores(kt)/exp(kt).
- fine-grained emission weaving: projection chain steps and the previous
  chunk's wo pieces distributed between attention k-tile units so in-order
  engine queues always have independent PE work while ACT runs exp.
- DMA dispatch spread across sequencers (SP: x/xbar, ACT: weights,
  Pool: output stores); x is staged host-side as 8KB-contiguous partition
  rows so each chunk is one 128-descriptor DMA.
- attention chunk order ends on the 4-k-tile chunk (b1,qc0) so the long
  (b1,qc3) chunk's normalize + wo drain under the tail chunk's work.
"""
import numpy as np

import concourse.bass as bass
import concourse.tile as tile
from concourse import bacc, mybir
from concourse.bass_utils import run_bass_kernel_spmd

B, S, D = 2, 2048, 1024
H, HD = 16, 64
NCORES = 8
SF = B * S              # 4096 flattened rows
CH = 512                # column chunk for matmuls
KT = 128                # k-tile (keys per tile)
NCH = SF // CH          # 8 projection chunks

F32 = mybir.dt.float32
F16 = mybir.dt.float16

import os
CFG_ORDER = os.environ.get("K_ORDER", "natural")      # natural | reorder
CFG_MASK = os.environ.get("K_MASK", "pool")           # pool | dve
CFG_WEAVE = os.environ.get("K_WEAVE", "early")        # early | even
CFG_EVAC = os.environ.get("K_EVAC", "mix")            # act | dve | mix
CFG_CHAIN = os.environ.get("K_CHAIN", "qvk")          # qkv | vkq | qvk

_cache = {}


def _emit_body(nc, tc, io, rep):
    xt, wqt, wkt, wvt, wot, outp = io
    Exp = mybir.ActivationFunctionType.Exp
    r_ = f"r{rep}_"

    with tc.tile_pool(name=r_ + "persist", bufs=1) as persist, \
         tc.tile_pool(name=r_ + "pj_ps", bufs=1, space="PSUM") as pj_ps, \
         tc.tile_pool(name=r_ + "sc_ps", bufs=2, space="PSUM") as sc_ps, \
         tc.tile_pool(name=r_ + "out_ps", bufs=1, space="PSUM") as out_ps, \
         tc.tile_pool(name=r_ + "trwo_ps", bufs=1, space="PSUM") as trwo_ps, \
         tc.tile_pool(name=r_ + "xt_p", bufs=3) as xt_p, \
         tc.tile_pool(name=r_ + "vt_p", bufs=2) as vt_p, \
         tc.tile_pool(name=r_ + "vs_p", bufs=2) as vs_p, \
         tc.tile_pool(name=r_ + "exp_p", bufs=6) as exp_p, \
         tc.tile_pool(name=r_ + "sums_p", bufs=4) as sums_p, \
         tc.tile_pool(name=r_ + "stg_p", bufs=4) as stg_p:

        qT = persist.tile([128, SF], F16)       # [pair-dim d, s]
        kT = persist.tile([128, SF], F16)
        vN = persist.tile([128, 2, 2 * S // KT, 65], F16)  # [s%128, hp, s-tile, v|1]
        oT = persist.tile([128, SF], F16)       # normalized attn out, T
        wq_s = persist.tile([128, 8, 128], F16)
        wk_s = persist.tile([128, 8, 128], F16)
        wv_s = persist.tile([128, 8, 128], F16)
        wo_s = persist.tile([128, D], F16)

        xt_r = xt.ap()          # [NCH, 128(p), 8(t), CH]

        xti = {}

        def xti_dma(sc, split=1):
            t = xt_p.tile([128, 8, CH], F16, name=f"xti_{rep}_{sc}", tag="xti")
            xti[sc] = t
            step = 8 // split
            for i in range(split):
                nc.sync.dma_start(t[:, i * step:(i + 1) * step, :],
                                  xt_r[sc, :, i * step:(i + 1) * step, :])

        # startup DMAs: weights from the ACT queue, x from SP in parallel
        nc.scalar.dma_start(wq_s[:], wqt.ap())
        xti_dma(0, split=2)
        nc.scalar.dma_start(wk_s[:], wkt.ap())
        nc.scalar.dma_start(wv_s[:], wvt.ap())
        xti_dma(1, split=2)
        nc.scalar.dma_start(wo_s[:], wot.ap())

        # ones columns (softmax denominator trick) + act table warmup
        nc.vector.memset(
            vN[:, :, :, 64:65].rearrange("p a b c -> p (a b c)"), 1.0)
        warm = sums_p.tile([128, 1], F16, name=f"warm_{rep}", tag="warm")
        nc.scalar.activation(warm[:], vN[:, 0, 0, 64:65], Exp, scale=0.125)
        mask_tile = None
        if CFG_MASK == "dve":
            mask_tile = persist.tile([128, 128], F32)
            nc.gpsimd.memset(mask_tile[:], 0.0)
            # mask_tile[p, c] = p <= c ? 0 : -1e38  (additive causal mask)
            nc.gpsimd.affine_select(
                out=mask_tile[:], in_=mask_tile[:],
                compare_op=mybir.AluOpType.is_ge,
                fill=-1.0e38, base=0, channel_multiplier=-1,
                pattern=[[1, 128]])

        W = {"q": wq_s, "k": wk_s, "v": wv_s}

        def proj_pieces(sc):
            """Emission closures projecting s-chunk sc into qT/kT/vN.
            First 10 pieces are the prefetch + q chain + q evac (the part a
            front-loaded attention chunk depends on)."""
            pieces = []
            st = {}
            col = slice(sc * CH, (sc + 1) * CH)

            def mk_mm(ch, t):
                def f():
                    if t == 0:
                        st[ch] = pj_ps.tile([128, CH], F32, tag="pj",
                                            name=f"ps{ch}_{rep}_{sc}")
                    nc.tensor.matmul(st[ch][:], W[ch][:, t, :], xti[sc][:, t, :],
                                     start=(t == 0), stop=(t == 7),
                                     skip_group_check=True)
                return f

            def qfin():
                if CFG_EVAC == "act":
                    nc.scalar.copy(qT[:, col], st["q"][:])
                else:
                    nc.vector.tensor_copy(qT[:, col], st["q"][:])

            def kfin():
                if CFG_EVAC in ("act", "mix"):
                    nc.scalar.copy(kT[:, col], st["k"][:])
                else:
                    nc.vector.tensor_copy(kT[:, col], st["k"][:])

            def vfin():
                vts = vt_p.tile([128, CH], F16, name=f"vts_{rep}_{sc}", tag="vts")
                nc.vector.tensor_copy(vts[:], st["v"][:])
                vstg = vs_p.tile([128, 4, 128], F16, name=f"vstg_{rep}_{sc}",
                                 tag="vstg")
                nc.sync.dma_start_transpose(vstg[:], vts[:])
                # vstg[p, t, hp*64+c] = v[dim hp*64+c, key t*128+p]
                nc.vector.tensor_copy(
                    vN[:, :, 4 * sc:4 * sc + 4, 0:64],
                    vstg[:].rearrange("p t (h c) -> p h t c", h=2))

            chain_fins = {"q": qfin, "k": kfin, "v": vfin}
            for ci, ch in enumerate(CFG_CHAIN):
                for t in range(8):
                    pieces.append(mk_mm(ch, t))
                pieces.append(chain_fins[ch])
                if ci == 0 and sc + 2 < NCH:
                    pieces.append(lambda sc=sc: xti_dma(sc + 2))
            return pieces

        def attn_units(b, qc, tail=False):
            """(kt units + normalize, wo pieces) for q-chunk qc of batch b."""
            bcol = b * S
            qsl = slice(bcol + qc * CH, bcol + (qc + 1) * CH)
            nkt = 4 * (qc + 1)
            st = {}

            def scores_exp(kt):
                r = kt * KT - qc * CH
                r0 = max(r, 0)
                ps_m = sc_ps.tile([128, 2, CH], F32, tag="ps_s",
                                  name=f"ps_m_{rep}_{b}_{qc}_{kt}")
                et = exp_p.tile([128, 2, CH], F16, tag="et",
                                name=f"et_{rep}_{b}_{qc}_{kt}")
                for hp in range(2):
                    hsl = slice(hp * 64, hp * 64 + 64)
                    nc.tensor.matmul(
                        ps_m[:, hp, r0:CH],
                        kT[hsl, bcol + kt * KT: bcol + (kt + 1) * KT],
                        qT[hsl, bcol + qc * CH + r0: bcol + (qc + 1) * CH],
                        start=True, stop=True)
                if CFG_MASK == "dve" and r >= 0:
                    for hp in range(2):
                        nc.vector.tensor_add(ps_m[:, hp, r:r + 128],
                                             ps_m[:, hp, r:r + 128],
                                             mask_tile[:])
                nc.scalar.activation(et[:, :, r0:CH], ps_m[:, :, r0:CH],
                                     Exp, scale=0.125)
                if CFG_MASK == "pool" and r >= 0:
                    # zero the key>query triangle of the diagonal 128 cols
                    for hp in range(2):
                        nc.gpsimd.affine_select(
                            out=et[:, hp, r:r + 128],
                            in_=et[:, hp, r:r + 128],
                            compare_op=mybir.AluOpType.is_ge,
                            fill=0.0, base=0, channel_multiplier=-1,
                            pattern=[[1, 128]])
                st[("et", kt)] = (et, r0)

            def av(kt):
                et, r0 = st.pop(("et", kt))
                if kt == 0:
                    st["o"] = [out_ps.tile([65, CH], F32, tag=f"ps_o{i}",
                                           name=f"ps_o{i}_{rep}_{b}_{qc}")
                               for i in range(2)]
                for hp in range(2):
                    nc.tensor.matmul(
                        st["o"][hp][:, r0:CH],
                        vN[:, hp, b * (S // KT) + kt, :],
                        et[:, hp, r0:CH],
                        start=(kt == 0), stop=(kt == nkt - 1),
                        skip_group_check=True)

            def norm_block(st4):
                # queries in 128-block st4 get no contribution from k-tiles
                # past their diagonal, so they are final after av(4qc+st4)
                csl = slice(st4 * 128, (st4 + 1) * 128)
                q0 = bcol + qc * CH + st4 * 128
                for hp in range(2):
                    rrow = sums_p.tile([1, 128], F32, tag="rrow",
                                       name=f"rrow_{rep}_{b}_{qc}_{st4}_{hp}")
                    nc.vector.reciprocal(rrow[:], st["o"][hp][64:65, csl])
                    bc = sums_p.tile([64, 128], F32, tag="bc",
                                     name=f"bc_{rep}_{b}_{qc}_{st4}_{hp}")
                    nc.gpsimd.partition_broadcast(bc[:], rrow[0:1, :])
                    nc.vector.tensor_mul(
                        oT[hp * 64: hp * 64 + 64, q0:q0 + 128],
                        st["o"][hp][0:64, csl], bc[:])

            def norm_all():
                for hp in range(2):
                    rrow = sums_p.tile([1, CH], F32, tag="rrowf",
                                       name=f"rrow_{rep}_{b}_{qc}_{hp}")
                    nc.vector.reciprocal(rrow[:], st["o"][hp][64:65, :])
                    bc = sums_p.tile([64, CH], F32, tag="bcf",
                                     name=f"bc_{rep}_{b}_{qc}_{hp}")
                    nc.gpsimd.partition_broadcast(bc[:], rrow[0:1, :])
                    nc.vector.tensor_mul(
                        oT[hp * 64: hp * 64 + 64, qsl],
                        st["o"][hp][0:64, :], bc[:])

            stgs = {}
            OUTM = int(os.environ.get("K_OUTM", "2"))

            def mk_wo(st4, chn):
                soff = bcol + qc * CH + st4 * 128
                g = st4 // OUTM

                def f():
                    if st4 % OUTM == 0 and chn == 0:
                        stgs[g] = stg_p.tile([128, OUTM, D], F16, tag="stg",
                                             name=f"stg_{rep}_{b}_{qc}_{g}")
                    # tail: alternate PSUM slots with the (now idle) pj bank
                    # so back-to-back psf matmuls don't wait on evacuation
                    pool = pj_ps if (tail and chn == 1) else trwo_ps
                    ptag = "pj" if (tail and chn == 1) else "trwo"
                    psf = pool.tile([128, CH], F32, tag=ptag,
                                    name=f"psf_{rep}_{b}_{qc}_{st4}_{chn}")
                    nc.tensor.matmul(psf[:],
                                     oT[:, soff: soff + 128],
                                     wo_s[:, chn * CH:(chn + 1) * CH],
                                     start=True, stop=True)
                    # tail chunk: split evacuations across DVE and ACT
                    if tail and (st4 + chn) % 2 == 1:
                        nc.scalar.copy(
                            stgs[g][:, st4 % OUTM, chn * CH:(chn + 1) * CH],
                            psf[:])
                    else:
                        nc.vector.tensor_copy(
                            stgs[g][:, st4 % OUTM, chn * CH:(chn + 1) * CH],
                            psf[:])
                    if st4 % OUTM == OUTM - 1 and chn == 1:
                        g0 = bcol + qc * CH + g * OUTM * 128
                        nc.sync.dma_start(
                            outp.ap()[g0: g0 + OUTM * 128, :]
                                .rearrange("(a p) d -> p a d", a=OUTM),
                            stgs[g][:])
                return f

            if tail:
                # pipeline the tail: each query 128-block is normalized as
                # soon as its diagonal k-tile's av lands; its wo runs one
                # unit later so the normalize chain hides under the next exp
                def mk_unit(kt):
                    def f(filler):
                        scores_exp(kt)
                        filler()
                        if kt > 0:
                            av(kt - 1)
                            if kt - 1 >= 4 * qc:
                                norm_block(kt - 1 - 4 * qc)
                            if kt - 2 >= 4 * qc:
                                s4 = kt - 2 - 4 * qc
                                mk_wo(s4, 0)()
                                mk_wo(s4, 1)()
                    return f
                units = [mk_unit(kt) for kt in range(nkt)]

                def last_unit(filler):
                    filler()
                    av(nkt - 1)
                    norm_block(3)
                    mk_wo(2, 0)()
                    mk_wo(2, 1)()
                    mk_wo(3, 0)()
                    mk_wo(3, 1)()
                units.append(last_unit)
                return units, []
            else:
                def mk_unit(kt):
                    def f(filler):
                        scores_exp(kt)
                        filler()
                        if kt > 0:
                            av(kt - 1)
                    return f
                units = [mk_unit(kt) for kt in range(nkt)]

                def last_unit(filler):
                    filler()
                    av(nkt - 1)
                    norm_all()
                units.append(last_unit)
                wo_pieces = [mk_wo(st4, chn) for st4 in range(4)
                             for chn in range(2)]
                return units, wo_pieces

        def weave(units, side_proj, side_wo, front):
            """Emit units, feeding side pieces to each unit's filler slot
            (between its scores and its stalling av). Proj pieces land within
            the first (n-2) units (v pipeline latency before the diagonal
            av); wo pieces spread over all units."""
            for p in front:
                p()
            n = len(units)
            np_, nw = len(side_proj), len(side_wo)
            nspan = max(1, n - 2) if (CFG_WEAVE == "early" and n > 2) else n
            state = {"pi": 0, "wi": 0}

            def filler_for(j):
                def filler():
                    wantp = min(((j + 1) * np_) // nspan, np_)
                    wantw = ((j + 1) * nw) // n
                    while state["pi"] < wantp:
                        side_proj[state["pi"]]()
                        state["pi"] += 1
                    while state["wi"] < wantw:
                        side_wo[state["wi"]]()
                        state["wi"] += 1
                return filler

            for j, u in enumerate(units):
                u(filler_for(j))
            while state["pi"] < np_:
                side_proj[state["pi"]]()
                state["pi"] += 1
            while state["wi"] < nw:
                side_wo[state["wi"]]()
                state["wi"] += 1

        # attention slot order: end on the short (b1,qc0) chunk so the big
        # (b1,qc3) chunk's normalize+wo drain under it. b1 slots 4..6 then
        # need their OWN query chunk's projection front-loaded.
        if CFG_ORDER == "reorder":
            chunks = [(0, 0), (0, 1), (0, 2), (0, 3), (1, 1), (1, 2), (1, 3), (1, 0)]
            front_slots = (4, 5, 6)
        else:
            chunks = [(b, qc) for b in range(B) for qc in range(4)]
            front_slots = ()
        for p in proj_pieces(0):
            p()
        pending = list(proj_pieces(1)) if NCH > 1 else []
        prev_wo = []
        for i, (b, qc) in enumerate(chunks):
            units, wo_pieces = attn_units(b, qc, tail=(i == len(chunks) - 1))
            if i in front_slots:
                front, rest = pending[:10], pending[10:]
            else:
                front, rest = [], pending
            weave(units, rest, prev_wo, front)
            pending = list(proj_pieces(i + 2)) if i + 2 < NCH else []
            prev_wo = wo_pieces
        for p in prev_wo:
            p()


def _build(repeats=1):
    nc = bacc.Bacc("TRN2", target_bir_lowering=False, debug=False)
    xt = nc.dram_tensor("xt", [NCH, 128, 8, CH], F16, kind="ExternalInput")
    wqt = nc.dram_tensor("wqt", [128, 8, 128], F16, kind="ExternalInput")
    wkt = nc.dram_tensor("wkt", [128, 8, 128], F16, kind="ExternalInput")
    wvt = nc.dram_tensor("wvt", [128, 8, 128], F16, kind="ExternalInput")
    wot = nc.dram_tensor("wot", [128, D], F16, kind="ExternalInput")
    outp = nc.dram_tensor("outp", [SF, D], F16, kind="ExternalOutput")
    io = (xt, wqt, wkt, wvt, wot, outp)

    with tile.TileContext(nc) as tc:
        for rep in range(repeats):
            _emit_body(nc, tc, io, rep)
    nc.compile()
    return nc


def make_in_maps(x, wq, wk, wv, wo):
    # xt_arr[sc, p, t, s] = x[sc*CH + s, t*128 + p] — every partition row of
    # a chunk is 8KB contiguous, so one chunk loads as one 128-descriptor DMA
    xt = np.ascontiguousarray(
        x.reshape(SF // CH, CH, 8, 128).transpose(0, 3, 2, 1)).astype(np.float16)

    def wslice(wT):
        # device layout w_s[p, t, m] = wT[t*128+p, m]; wT is [1024 feat, 128 out]
        return np.ascontiguousarray(
            wT.reshape(8, 128, 128).transpose(1, 0, 2)).astype(np.float16)

    in_maps = []
    for c in range(NCORES):
        rows = slice(c * 128, (c + 1) * 128)
        in_maps.append({
            "xt": xt,
            "wqt": wslice(wq[rows, :].T),
            "wkt": wslice(wk[rows, :].T),
            "wvt": wslice(wv[rows, :].T),
            "wot": np.ascontiguousarray(wo[:, rows].T).astype(np.float16),
        })
    return in_maps


def _make_runner(nc):
    """Build a cached jitted PJRT runner. xt is replicated (same data on
    every core); weight slices are sharded per core; outputs unsharded on
    host."""
    import jax
    from jax.sharding import Mesh, PartitionSpec, NamedSharding
    try:
        from jax.experimental.shard_map import shard_map
    except ImportError:
        shard_map = jax.shard_map
    from concourse.bass2jax import (_bass_exec_p, install_neuronx_cc_hook,
                                    partition_id_tensor)

    install_neuronx_cc_hook()
    pname = nc.partition_id_tensor.name if nc.partition_id_tensor else None
    in_names, out_names, out_avals, zero_shapes = [], [], [], []
    for alloc in nc.m.functions[0].allocations:
        if not isinstance(alloc, mybir.MemoryLocationSet):
            continue
        name = alloc.memorylocations[0].name
        if alloc.kind == "ExternalInput":
            if name != pname:
                in_names.append(name)
        elif alloc.kind == "ExternalOutput":
            out_names.append(name)
            shape = tuple(alloc.tensor_shape)
            dtype = mybir.dt.np(alloc.dtype)
            out_avals.append(jax.core.ShapedArray(shape, dtype))
            zero_shapes.append((shape, dtype))
    all_in_names = in_names + out_names
    if pname is not None:
        all_in_names = all_in_names + [pname]

    def _body(*args):
        operands = list(args)
        if pname is not None:
            operands.append(partition_id_tensor())
        return tuple(_bass_exec_p.bind(
            *operands,
            out_avals=tuple(out_avals),
            in_names=tuple(all_in_names),
            out_names=tuple(out_names),
            lowering_input_output_aliases=(),
            sim_require_finite=True,
            sim_require_nnan=True,
            nc=nc,
        ))

    devices = jax.devices()[:NCORES]
    mesh = Mesh(np.asarray(devices), ("core",))
    shard = PartitionSpec("core")
    repl = PartitionSpec()
    REPLICATED = ("xt",)
    in_specs = tuple(repl if n in REPLICATED else shard for n in in_names) \
        + (shard,) * len(out_names)
    sharded = jax.jit(
        shard_map(_body, mesh=mesh, in_specs=in_specs,
                  out_specs=(shard,) * len(out_names), check_rep=False),
        keep_unused=True)
    zeros = [jax.device_put(np.zeros((NCORES * s[0], *s[1:]), d),
                            NamedSharding(mesh, shard))
             for (s, d) in zero_shapes]
    jax.block_until_ready(zeros)

    def run(in_maps):
        args = []
        for n in in_names:
            if n in REPLICATED:
                args.append(jax.device_put(np.asarray(in_maps[0][n]),
                                           NamedSharding(mesh, repl)))
            else:
                args.append(jax.device_put(
                    np.concatenate([np.asarray(m[n]) for m in in_maps], axis=0),
                    NamedSharding(mesh, shard)))
        outs = sharded(*args, *zeros)
        return [
            {n: np.asarray(outs[i]).reshape(NCORES, *out_avals[i].shape)[c]
             for i, n in enumerate(out_names)}
            for c in range(NCORES)
        ]

    return run


def kernel(x, wq, wk, wv, wo):
    x = np.asarray(x, dtype=np.float32)
    wq = np.asarray(wq, dtype=np.float32)
    wk = np.asarray(wk, dtype=np.float32)
    wv = np.asarray(wv, dtype=np.float32)
    wo = np.asarray(wo, dtype=np.float32)

    if "nc" not in _cache:
        _cache["nc"] = _build()
    nc = _cache["nc"]
    in_maps = make_in_maps(x, wq, wk, wv, wo)

    try:
        if "run" not in _cache:
            _cache["run"] = _make_runner(nc)
        results = _cache["run"](in_maps)
    except Exception:
        _cache.pop("run", None)
        results = run_bass_kernel_spmd(
            nc, in_maps, core_ids=list(range(NCORES))).results

    out = np.zeros((SF, D), dtype=np.float64)
    for r in results:
        out += r["outp"].astype(np.float64)
    return out.astype(np.float32).reshape(B, S, D)


# revision 31
# speedup vs baseline: 1.0380x; 1.0380x over previous
"""Causal multi-head attention (B=2, S=2048, D=1024, H=16) on 8 NeuronCores.

Sharding: head-parallel. Core c owns heads {2c, 2c+1} = a 128-wide slice of
the q/k/v projection output dims and of wo's input dim. Each core computes
attention for its 2 heads over both batch elements and a full-size partial
of the final projection; the host sums the 8 partials.

Design (v2):
- fp16 end-to-end (HBM tensors, SBUF operands; PSUM stays f32).
- scores computed transposed (scoresT[k, q]) so softmax probs come out
  k-partitioned and feed attn@v directly; a ones-column in the v tile emits
  the softmax denominators from the same matmul.
- causal mask applied post-exp on the (idle) Pool engine via affine_select.
- v tiles transposed via the DMA XBAR (dma_start_transpose), not PE.
- software-pipelined attention: av(kt-1) emitted after scores(kt)/exp(kt).
- fine-grained emission weaving: projection chain steps and the previous
  chunk's wo pieces distributed between attention k-tile units so in-order
  engine queues always have independent PE work while ACT runs exp.
- DMA dispatch spread across sequencers (SP: x/xbar, ACT: weights,
  Pool: output stores); x is staged host-side as 8KB-contiguous partition
  rows so each chunk is one 128-descriptor DMA.
- attention chunk order ends on the 4-k-tile chunk (b1,qc0) so the long
  (b1,qc3) chunk's normalize + wo drain under the tail chunk's work.
"""
import numpy as np

import concourse.bass as bass
import concourse.tile as tile
from concourse import bacc, mybir
from concourse.bass_utils import run_bass_kernel_spmd

B, S, D = 2, 2048, 1024
H, HD = 16, 64
NCORES = 8
SF = B * S              # 4096 flattened rows
CH = 512                # column chunk for matmuls
KT = 128                # k-tile (keys per tile)
NCH = SF // CH          # 8 projection chunks

F32 = mybir.dt.float32
F16 = mybir.dt.float16

import os
CFG_ORDER = os.environ.get("K_ORDER", "natural")      # natural | reorder
CFG_MASK = os.environ.get("K_MASK", "pool")           # pool | dve
CFG_WEAVE = os.environ.get("K_WEAVE", "early")        # early | even
CFG_EVAC = os.environ.get("K_EVAC", "mix")            # act | dve | mix
CFG_CHAIN = os.environ.get("K_CHAIN", "qvk")          # qkv | vkq | qvk

_cache = {}


def _emit_body(nc, tc, io, pools, rep):
    xt, wqt, wkt, wvt, wot, outp = io
    Exp = mybir.ActivationFunctionType.Exp
    r_ = f"r{rep}_"

    if True:
        (persist, pj_ps, sc_ps, out_ps, trwo_ps,
         xt_p, vt_p, vs_p, exp_p, sums_p, stg_p) = pools

        # persist pool has bufs=2: consecutive reps alternate slots, so a
        # rep's projections can start under the previous rep's attention tail
        qT = persist.tile([128, SF], F16, name=f"qT_{rep}", tag="qT")
        kT = persist.tile([128, SF], F16, name=f"kT_{rep}", tag="kT")
        vN = persist.tile([128, 2, 2 * S // KT, 65], F16,
                          name=f"vN_{rep}", tag="vN")
        oT = persist.tile([128, SF], F16, name=f"oT_{rep}", tag="oT")
        wq_s = persist.tile([128, 8, 128], F16, name=f"wq_{rep}", tag="wq")
        wk_s = persist.tile([128, 8, 128], F16, name=f"wk_{rep}", tag="wk")
        wv_s = persist.tile([128, 8, 128], F16, name=f"wv_{rep}", tag="wv")
        wo_s = persist.tile([128, D], F16, name=f"wo_{rep}", tag="wo")

        xt_r = xt.ap()          # [NCH, 128(p), 8(t), CH]

        xti = {}

        def xti_dma(sc, split=1):
            t = xt_p.tile([128, 8, CH], F16, name=f"xti_{rep}_{sc}", tag="xti")
            xti[sc] = t
            step = 8 // split
            for i in range(split):
                nc.sync.dma_start(t[:, i * step:(i + 1) * step, :],
                                  xt_r[sc, :, i * step:(i + 1) * step, :])

        # startup DMAs: weights from the ACT queue, x from SP in parallel
        nc.scalar.dma_start(wq_s[:], wqt.ap())
        xti_dma(0, split=2)
        nc.scalar.dma_start(wk_s[:], wkt.ap())
        nc.scalar.dma_start(wv_s[:], wvt.ap())
        xti_dma(1, split=2)
        nc.scalar.dma_start(wo_s[:], wot.ap())

        # ones columns (softmax denominator trick) + act table warmup
        nc.vector.memset(
            vN[:, :, :, 64:65].rearrange("p a b c -> p (a b c)"), 1.0)
        warm = sums_p.tile([128, 1], F16, name=f"warm_{rep}", tag="warm")
        nc.scalar.activation(warm[:], vN[:, 0, 0, 64:65], Exp, scale=0.125)
        mask_tile = None
        if CFG_MASK == "dve":
            mask_tile = persist.tile([128, 128], F32, name=f"mk_{rep}", tag="mk")
            nc.gpsimd.memset(mask_tile[:], 0.0)
            # mask_tile[p, c] = p <= c ? 0 : -1e38  (additive causal mask)
            nc.gpsimd.affine_select(
                out=mask_tile[:], in_=mask_tile[:],
                compare_op=mybir.AluOpType.is_ge,
                fill=-1.0e38, base=0, channel_multiplier=-1,
                pattern=[[1, 128]])

        W = {"q": wq_s, "k": wk_s, "v": wv_s}

        def proj_pieces(sc):
            """Emission closures projecting s-chunk sc into qT/kT/vN.
            First 10 pieces are the prefetch + q chain + q evac (the part a
            front-loaded attention chunk depends on)."""
            pieces = []
            st = {}
            col = slice(sc * CH, (sc + 1) * CH)

            def mk_mm(ch, t):
                def f():
                    if t == 0:
                        st[ch] = pj_ps.tile([128, CH], F32, tag="pj",
                                            name=f"ps{ch}_{rep}_{sc}")
                    nc.tensor.matmul(st[ch][:], W[ch][:, t, :], xti[sc][:, t, :],
                                     start=(t == 0), stop=(t == 7),
                                     skip_group_check=True)
                return f

            def qfin():
                if CFG_EVAC == "act":
                    nc.scalar.copy(qT[:, col], st["q"][:])
                else:
                    nc.vector.tensor_copy(qT[:, col], st["q"][:])

            def kfin():
                if CFG_EVAC in ("act", "mix"):
                    nc.scalar.copy(kT[:, col], st["k"][:])
                else:
                    nc.vector.tensor_copy(kT[:, col], st["k"][:])

            def vfin():
                vts = vt_p.tile([128, CH], F16, name=f"vts_{rep}_{sc}", tag="vts")
                nc.vector.tensor_copy(vts[:], st["v"][:])
                vstg = vs_p.tile([128, 4, 128], F16, name=f"vstg_{rep}_{sc}",
                                 tag="vstg")
                nc.sync.dma_start_transpose(vstg[:], vts[:])
                # vstg[p, t, hp*64+c] = v[dim hp*64+c, key t*128+p]
                nc.vector.tensor_copy(
                    vN[:, :, 4 * sc:4 * sc + 4, 0:64],
                    vstg[:].rearrange("p t (h c) -> p h t c", h=2))

            chain_fins = {"q": qfin, "k": kfin, "v": vfin}
            for ci, ch in enumerate(CFG_CHAIN):
                for t in range(8):
                    pieces.append(mk_mm(ch, t))
                pieces.append(chain_fins[ch])
                if ci == 0 and sc + 2 < NCH:
                    pieces.append(lambda sc=sc: xti_dma(sc + 2))
            return pieces

        def attn_units(b, qc, tail=False):
            """(kt units + normalize, wo pieces) for q-chunk qc of batch b."""
            bcol = b * S
            qsl = slice(bcol + qc * CH, bcol + (qc + 1) * CH)
            nkt = 4 * (qc + 1)
            st = {}

            def scores_exp(kt):
                r = kt * KT - qc * CH
                r0 = max(r, 0)
                ps_m = sc_ps.tile([128, 2, CH], F32, tag="ps_s",
                                  name=f"ps_m_{rep}_{b}_{qc}_{kt}")
                et = exp_p.tile([128, 2, CH], F16, tag="et",
                                name=f"et_{rep}_{b}_{qc}_{kt}")
                for hp in range(2):
                    hsl = slice(hp * 64, hp * 64 + 64)
                    nc.tensor.matmul(
                        ps_m[:, hp, r0:CH],
                        kT[hsl, bcol + kt * KT: bcol + (kt + 1) * KT],
                        qT[hsl, bcol + qc * CH + r0: bcol + (qc + 1) * CH],
                        start=True, stop=True)
                if CFG_MASK == "dve" and r >= 0:
                    for hp in range(2):
                        nc.vector.tensor_add(ps_m[:, hp, r:r + 128],
                                             ps_m[:, hp, r:r + 128],
                                             mask_tile[:])
                nc.scalar.activation(et[:, :, r0:CH], ps_m[:, :, r0:CH],
                                     Exp, scale=0.125)
                if CFG_MASK == "pool" and r >= 0:
                    # zero the key>query triangle of the diagonal 128 cols
                    for hp in range(2):
                        nc.gpsimd.affine_select(
                            out=et[:, hp, r:r + 128],
                            in_=et[:, hp, r:r + 128],
                            compare_op=mybir.AluOpType.is_ge,
                            fill=0.0, base=0, channel_multiplier=-1,
                            pattern=[[1, 128]])
                st[("et", kt)] = (et, r0)

            def av(kt):
                et, r0 = st.pop(("et", kt))
                if kt == 0:
                    st["o"] = [out_ps.tile([65, CH], F32, tag=f"ps_o{i}",
                                           name=f"ps_o{i}_{rep}_{b}_{qc}")
                               for i in range(2)]
                for hp in range(2):
                    nc.tensor.matmul(
                        st["o"][hp][:, r0:CH],
                        vN[:, hp, b * (S // KT) + kt, :],
                        et[:, hp, r0:CH],
                        start=(kt == 0), stop=(kt == nkt - 1),
                        skip_group_check=True)

            def norm_block(st4):
                # queries in 128-block st4 get no contribution from k-tiles
                # past their diagonal, so they are final after av(4qc+st4)
                csl = slice(st4 * 128, (st4 + 1) * 128)
                q0 = bcol + qc * CH + st4 * 128
                for hp in range(2):
                    rrow = sums_p.tile([1, 128], F32, tag="rrow",
                                       name=f"rrow_{rep}_{b}_{qc}_{st4}_{hp}")
                    nc.vector.reciprocal(rrow[:], st["o"][hp][64:65, csl])
                    bc = sums_p.tile([64, 128], F32, tag="bc",
                                     name=f"bc_{rep}_{b}_{qc}_{st4}_{hp}")
                    nc.gpsimd.partition_broadcast(bc[:], rrow[0:1, :])
                    nc.vector.tensor_mul(
                        oT[hp * 64: hp * 64 + 64, q0:q0 + 128],
                        st["o"][hp][0:64, csl], bc[:])

            def norm_all():
                for hp in range(2):
                    rrow = sums_p.tile([1, CH], F32, tag="rrowf",
                                       name=f"rrow_{rep}_{b}_{qc}_{hp}")
                    nc.vector.reciprocal(rrow[:], st["o"][hp][64:65, :])
                    bc = sums_p.tile([64, CH], F32, tag="bcf",
                                     name=f"bc_{rep}_{b}_{qc}_{hp}")
                    nc.gpsimd.partition_broadcast(bc[:], rrow[0:1, :])
                    nc.vector.tensor_mul(
                        oT[hp * 64: hp * 64 + 64, qsl],
                        st["o"][hp][0:64, :], bc[:])

            stgs = {}
            OUTM = int(os.environ.get("K_OUTM", "2"))

            def mk_wo(st4, chn):
                soff = bcol + qc * CH + st4 * 128
                g = st4 // OUTM

                def f():
                    if st4 % OUTM == 0 and chn == 0:
                        stgs[g] = stg_p.tile([128, OUTM, D], F16, tag="stg",
                                             name=f"stg_{rep}_{b}_{qc}_{g}")
                    # tail: alternate PSUM slots with the (now idle) pj bank
                    # so back-to-back psf matmuls don't wait on evacuation
                    pool = pj_ps if (tail and chn == 1) else trwo_ps
                    ptag = "pj" if (tail and chn == 1) else "trwo"
                    psf = pool.tile([128, CH], F32, tag=ptag,
                                    name=f"psf_{rep}_{b}_{qc}_{st4}_{chn}")
                    nc.tensor.matmul(psf[:],
                                     oT[:, soff: soff + 128],
                                     wo_s[:, chn * CH:(chn + 1) * CH],
                                     start=True, stop=True)
                    # tail chunk: split evacuations across DVE and ACT
                    if tail and (st4 + chn) % 2 == 1:
                        nc.scalar.copy(
                            stgs[g][:, st4 % OUTM, chn * CH:(chn + 1) * CH],
                            psf[:])
                    else:
                        nc.vector.tensor_copy(
                            stgs[g][:, st4 % OUTM, chn * CH:(chn + 1) * CH],
                            psf[:])
                    if st4 % OUTM == OUTM - 1 and chn == 1:
                        g0 = bcol + qc * CH + g * OUTM * 128
                        nc.sync.dma_start(
                            outp.ap()[g0: g0 + OUTM * 128, :]
                                .rearrange("(a p) d -> p a d", a=OUTM),
                            stgs[g][:])
                return f

            if tail:
                # pipeline the tail: each query 128-block is normalized as
                # soon as its diagonal k-tile's av lands; its wo runs one
                # unit later so the normalize chain hides under the next exp
                def mk_unit(kt):
                    def f(filler):
                        scores_exp(kt)
                        filler()
                        if kt > 0:
                            av(kt - 1)
                            if kt - 1 >= 4 * qc:
                                norm_block(kt - 1 - 4 * qc)
                            if kt - 2 >= 4 * qc:
                                s4 = kt - 2 - 4 * qc
                                mk_wo(s4, 0)()
                                mk_wo(s4, 1)()
                    return f
                units = [mk_unit(kt) for kt in range(nkt)]

                def last_unit(filler):
                    filler()
                    av(nkt - 1)
                    norm_block(3)
                    mk_wo(2, 0)()
                    mk_wo(2, 1)()
                    mk_wo(3, 0)()
                    mk_wo(3, 1)()
                units.append(last_unit)
                return units, []
            else:
                def mk_unit(kt):
                    def f(filler):
                        scores_exp(kt)
                        filler()
                        if kt > 0:
                            av(kt - 1)
                    return f
                units = [mk_unit(kt) for kt in range(nkt)]

                def last_unit(filler):
                    filler()
                    av(nkt - 1)
                    norm_all()
                units.append(last_unit)
                wo_pieces = [mk_wo(st4, chn) for st4 in range(4)
                             for chn in range(2)]
                return units, wo_pieces

        def weave(units, side_proj, side_wo, front):
            """Emit units, feeding side pieces to each unit's filler slot
            (between its scores and its stalling av). Proj pieces land within
            the first (n-2) units (v pipeline latency before the diagonal
            av); wo pieces spread over all units."""
            for p in front:
                p()
            n = len(units)
            np_, nw = len(side_proj), len(side_wo)
            nspan = max(1, n - 2) if (CFG_WEAVE == "early" and n > 2) else n
            state = {"pi": 0, "wi": 0}

            def filler_for(j):
                def filler():
                    wantp = min(((j + 1) * np_) // nspan, np_)
                    wantw = ((j + 1) * nw) // n
                    while state["pi"] < wantp:
                        side_proj[state["pi"]]()
                        state["pi"] += 1
                    while state["wi"] < wantw:
                        side_wo[state["wi"]]()
                        state["wi"] += 1
                return filler

            for j, u in enumerate(units):
                u(filler_for(j))
            while state["pi"] < np_:
                side_proj[state["pi"]]()
                state["pi"] += 1
            while state["wi"] < nw:
                side_wo[state["wi"]]()
                state["wi"] += 1

        # attention slot order: end on the short (b1,qc0) chunk so the big
        # (b1,qc3) chunk's normalize+wo drain under it. b1 slots 4..6 then
        # need their OWN query chunk's projection front-loaded.
        if CFG_ORDER == "reorder":
            chunks = [(0, 0), (0, 1), (0, 2), (0, 3), (1, 1), (1, 2), (1, 3), (1, 0)]
            front_slots = (4, 5, 6)
        else:
            chunks = [(b, qc) for b in range(B) for qc in range(4)]
            front_slots = ()
        for p in proj_pieces(0):
            p()
        pending = list(proj_pieces(1)) if NCH > 1 else []
        prev_wo = []
        for i, (b, qc) in enumerate(chunks):
            units, wo_pieces = attn_units(b, qc, tail=(i == len(chunks) - 1))
            if i in front_slots:
                front, rest = pending[:10], pending[10:]
            else:
                front, rest = [], pending
            weave(units, rest, prev_wo, front)
            pending = list(proj_pieces(i + 2)) if i + 2 < NCH else []
            prev_wo = wo_pieces
        for p in prev_wo:
            p()


def _build(repeats=1):
    nc = bacc.Bacc("TRN2", target_bir_lowering=False, debug=False)
    xt = nc.dram_tensor("xt", [NCH, 128, 8, CH], F16, kind="ExternalInput")
    wqt = nc.dram_tensor("wqt", [128, 8, 128], F16, kind="ExternalInput")
    wkt = nc.dram_tensor("wkt", [128, 8, 128], F16, kind="ExternalInput")
    wvt = nc.dram_tensor("wvt", [128, 8, 128], F16, kind="ExternalInput")
    wot = nc.dram_tensor("wot", [128, D], F16, kind="ExternalInput")
    outp = nc.dram_tensor("outp", [SF, D], F16, kind="ExternalOutput")
    io = (xt, wqt, wkt, wvt, wot, outp)

    with tile.TileContext(nc) as tc:
        with tc.tile_pool(name="persist", bufs=2) as persist, \
             tc.tile_pool(name="pj_ps", bufs=1, space="PSUM") as pj_ps, \
             tc.tile_pool(name="sc_ps", bufs=2, space="PSUM") as sc_ps, \
             tc.tile_pool(name="out_ps", bufs=1, space="PSUM") as out_ps, \
             tc.tile_pool(name="trwo_ps", bufs=1, space="PSUM") as trwo_ps, \
             tc.tile_pool(name="xt_p", bufs=3) as xt_p, \
             tc.tile_pool(name="vt_p", bufs=2) as vt_p, \
             tc.tile_pool(name="vs_p", bufs=2) as vs_p, \
             tc.tile_pool(name="exp_p", bufs=6) as exp_p, \
             tc.tile_pool(name="sums_p", bufs=4) as sums_p, \
             tc.tile_pool(name="stg_p", bufs=4) as stg_p:
            pools = (persist, pj_ps, sc_ps, out_ps, trwo_ps,
                     xt_p, vt_p, vs_p, exp_p, sums_p, stg_p)
            for rep in range(repeats):
                _emit_body(nc, tc, io, pools, rep)
    nc.compile()
    return nc


def make_in_maps(x, wq, wk, wv, wo):
    # xt_arr[sc, p, t, s] = x[sc*CH + s, t*128 + p] — every partition row of
    # a chunk is 8KB contiguous, so one chunk loads as one 128-descriptor DMA
    xt = np.ascontiguousarray(
        x.reshape(SF // CH, CH, 8, 128).transpose(0, 3, 2, 1)).astype(np.float16)

    def wslice(wT):
        # device layout w_s[p, t, m] = wT[t*128+p, m]; wT is [1024 feat, 128 out]
        return np.ascontiguousarray(
            wT.reshape(8, 128, 128).transpose(1, 0, 2)).astype(np.float16)

    in_maps = []
    for c in range(NCORES):
        rows = slice(c * 128, (c + 1) * 128)
        in_maps.append({
            "xt": xt,
            "wqt": wslice(wq[rows, :].T),
            "wkt": wslice(wk[rows, :].T),
            "wvt": wslice(wv[rows, :].T),
            "wot": np.ascontiguousarray(wo[:, rows].T).astype(np.float16),
        })
    return in_maps


def _make_runner(nc):
    """Build a cached jitted PJRT runner. xt is replicated (same data on
    every core); weight slices are sharded per core; outputs unsharded on
    host."""
    import jax
    from jax.sharding import Mesh, PartitionSpec, NamedSharding
    try:
        from jax.experimental.shard_map import shard_map
    except ImportError:
        shard_map = jax.shard_map
    from concourse.bass2jax import (_bass_exec_p, install_neuronx_cc_hook,
                                    partition_id_tensor)

    install_neuronx_cc_hook()
    pname = nc.partition_id_tensor.name if nc.partition_id_tensor else None
    in_names, out_names, out_avals, zero_shapes = [], [], [], []
    for alloc in nc.m.functions[0].allocations:
        if not isinstance(alloc, mybir.MemoryLocationSet):
            continue
        name = alloc.memorylocations[0].name
        if alloc.kind == "ExternalInput":
            if name != pname:
                in_names.append(name)
        elif alloc.kind == "ExternalOutput":
            out_names.append(name)
            shape = tuple(alloc.tensor_shape)
            dtype = mybir.dt.np(alloc.dtype)
            out_avals.append(jax.core.ShapedArray(shape, dtype))
            zero_shapes.append((shape, dtype))
    all_in_names = in_names + out_names
    if pname is not None:
        all_in_names = all_in_names + [pname]

    def _body(*args):
        operands = list(args)
        if pname is not None:
            operands.append(partition_id_tensor())
        return tuple(_bass_exec_p.bind(
            *operands,
            out_avals=tuple(out_avals),
            in_names=tuple(all_in_names),
            out_names=tuple(out_names),
            lowering_input_output_aliases=(),
            sim_require_finite=True,
            sim_require_nnan=True,
            nc=nc,
        ))

    devices = jax.devices()[:NCORES]
    mesh = Mesh(np.asarray(devices), ("core",))
    shard = PartitionSpec("core")
    repl = PartitionSpec()
    REPLICATED = ("xt",)
    in_specs = tuple(repl if n in REPLICATED else shard for n in in_names) \
        + (shard,) * len(out_names)
    sharded = jax.jit(
        shard_map(_body, mesh=mesh, in_specs=in_specs,
                  out_specs=(shard,) * len(out_names), check_rep=False),
        keep_unused=True)
    zeros = [jax.device_put(np.zeros((NCORES * s[0], *s[1:]), d),
                            NamedSharding(mesh, shard))
             for (s, d) in zero_shapes]
    jax.block_until_ready(zeros)

    def run(in_maps):
        args = []
        for n in in_names:
            if n in REPLICATED:
                args.append(jax.device_put(np.asarray(in_maps[0][n]),
                                           NamedSharding(mesh, repl)))
            else:
                args.append(jax.device_put(
                    np.concatenate([np.asarray(m[n]) for m in in_maps], axis=0),
                    NamedSharding(mesh, shard)))
        outs = sharded(*args, *zeros)
        return [
            {n: np.asarray(outs[i]).reshape(NCORES, *out_avals[i].shape)[c]
             for i, n in enumerate(out_names)}
            for c in range(NCORES)
        ]

    return run


def kernel(x, wq, wk, wv, wo):
    x = np.asarray(x, dtype=np.float32)
    wq = np.asarray(wq, dtype=np.float32)
    wk = np.asarray(wk, dtype=np.float32)
    wv = np.asarray(wv, dtype=np.float32)
    wo = np.asarray(wo, dtype=np.float32)

    if "nc" not in _cache:
        _cache["nc"] = _build()
    nc = _cache["nc"]
    in_maps = make_in_maps(x, wq, wk, wv, wo)

    try:
        if "run" not in _cache:
            _cache["run"] = _make_runner(nc)
        results = _cache["run"](in_maps)
    except Exception:
        _cache.pop("run", None)
        results = run_bass_kernel_spmd(
            nc, in_maps, core_ids=list(range(NCORES))).results

    out = np.zeros((SF, D), dtype=np.float64)
    for r in results:
        out += r["outp"].astype(np.float64)
    return out.astype(np.float32).reshape(B, S, D)
